# revision 16
# baseline (speedup 1.0000x reference)
"""Trainium2 Bass kernel for the DiffsolClassifier model (v2).

Network (per image, NCHW fp32):
    z1 = relu(conv2d(x, W1, b1, k=3, s=2, p=1))   # [8,14,14]
    z2 = relu(conv2d(z1, W2, b2, k=3, s=2, p=1))  # [16,7,7]
    t  = flatten(z2) @ Wfc.T + bfc                # [1]
    p  = clip(1 - exp(-(softplus(t) + 1e-3)), 1e-6, 1-1e-6)
       = 1 - k*sigmoid(-t),  k = exp(-1e-3)       (clip is a no-op)

Sharding: pure data parallel, batch 65536 split 8192/core across 8 cores.

Per-core mapping (16 outer tiles x 512 images), fp16 data / fp32 PSUM:
  - Host stages x as overlapping 84-pixel conv1 windows, pixel-major:
    xw[t, h, ...] rows (p, oi) with pixel 56*oi-28+p (zeros out of
    range). Two plain contiguous DMAs per tile -> SBUF [84,14,512] fp16.
  - conv1: 14 matmuls per tile with ONE shared stationary W1win [84,112]
    (W1win[28*di + 2*oj-1+dj, co*14+oj]); bias+relu evictions to z1
    [112, 14, 512] fp16 alternate across ACT/DVE.
  - conv2: banded tap mats [112,112] x 3; rows 0..6 accumulate 2-3 taps
    in PSUM; bias+relu eviction to z2 [112, 7, 512] fp16.
  - fc: per-row stationary padded to [112,128] (negated Wfc in col 0) so
    every matmul keeps the same (128,128) PE tiling - no PE tile-config
    switches. fc matmuls and the per-tile sigmoid+affine+store epilogue
    are deferred through a pending queue into later matmul slots so the
    PE never waits on an in-flight eviction.
  - epilogue: ACT sigmoid [1,512] (sigma(-t) trick), GPSIMD affine
    p = 1 - k*sigma, SP store. Weight loads ride the GPSIMD queue.
"""

import numpy as np

B = 65536
NCORES = 8
BS = B // NCORES  # 8192 images per core
TN = 512          # images per outer tile
NT = BS // TN     # 16 outer tiles

KDEC = float(np.exp(np.float32(-0.001)))

# set by test.py for profiling; harness leaves these alone
TRACE = False
LAST_EXEC_NS = None
LAST_PROFILE_JSON = None


def _build_weight_mats(W1, b1, W2, b2, Wfc):
    """Host-side restructuring of the tiny conv/fc weights."""
    W1 = np.asarray(W1, np.float32).reshape(8, 1, 3, 3)
    W2 = np.asarray(W2, np.float32).reshape(16, 8, 3, 3)
    Wfc = np.asarray(Wfc, np.float32).reshape(1, 784)

    # W1win[w, (co,oj)] over an 84-pixel window, w = 28*di + (2*oj-1+dj)
    W1win = np.zeros((84, 112), np.float32)
    for co in range(8):
        for oj in range(14):
            m = co * 14 + oj
            for di in range(3):
                for dj in range(3):
                    j = 2 * oj - 1 + dj
                    if 0 <= j < 28:
                        W1win[28 * di + j, m] = W1[co, 0, di, dj]

    # conv2 tap matrices: W2r[di][(ci,j), (co2,oj2)]
    W2r = np.zeros((3, 112, 112), np.float32)
    for di in range(3):
        for co in range(16):
            for oj in range(7):
                m = co * 7 + oj
                for ci in range(8):
                    for dj in range(3):
                        j = 2 * oj - 1 + dj
                        if 0 <= j < 14:
                            W2r[di, ci * 14 + j, m] = W2[co, ci, di, dj]

    # fc columns per z2 row, negated (p = 1 - k*sigmoid(-t) trick)
    wfc = np.zeros((112, 7), np.float32)
    for co in range(16):
        for i2 in range(7):
            for oj in range(7):
                wfc[co * 7 + oj, i2] = -Wfc[0, co * 49 + i2 * 7 + oj]

    b1col = np.repeat(np.asarray(b1, np.float32), 14).reshape(112, 1)
    b2col = np.repeat(np.asarray(b2, np.float32), 7).reshape(112, 1)
    return W1win, W2r, wfc, b1col, b2col


def _build_nc(nt_tiles):
    import concourse.bacc as bacc
    import concourse.bass as bass
    import concourse.mybir as mybir
    import concourse.tile as tile

    f32 = mybir.dt.float32
    f16 = mybir.dt.float16
    AF = mybir.ActivationFunctionType
    OP = mybir.AluOpType
    bs = nt_tiles * TN
    ngrp = (nt_tiles + 3) // 4

    nc = bacc.Bacc(None)
    xw_d = nc.declare_dram_parameter("xw", [nt_tiles, 2, 588, TN], f16,
                                     isOutput=False)
    w1_d = nc.declare_dram_parameter("w1win", [84, 112], f16, isOutput=False)
    w2r0_d = nc.declare_dram_parameter("w2r0", [112, 112], f16, isOutput=False)
    w2r1_d = nc.declare_dram_parameter("w2r1", [112, 112], f16, isOutput=False)
    w2r2_d = nc.declare_dram_parameter("w2r2", [112, 112], f16, isOutput=False)
    wfc_d = nc.declare_dram_parameter("wfcneg", [112, 7], f16, isOutput=False)
    b1_d = nc.declare_dram_parameter("b1col", [112, 1], f32, isOutput=False)
    b2_d = nc.declare_dram_parameter("b2col", [112, 1], f32, isOutput=False)
    bfc_d = nc.declare_dram_parameter("bfcneg", [128, 1], f32, isOutput=False)
    y_d = nc.declare_dram_parameter("y", [bs], f32, isOutput=True)

    with tile.TileContext(nc) as tc:
        with (
            tc.tile_pool(name="const", bufs=1) as const,
            tc.tile_pool(name="xt_pool", bufs=4) as xt_pool,
            tc.tile_pool(name="z1_pool", bufs=3) as z1_pool,
            tc.tile_pool(name="z2_pool", bufs=2) as z2_pool,
            tc.tile_pool(name="y_pool", bufs=2) as y_pool,
            tc.tile_pool(name="c1_psum", bufs=5, space="PSUM") as c1_pool,
            tc.tile_pool(name="c2_psum", bufs=2, space="PSUM") as c2_pool,
            tc.tile_pool(name="fc_psum", bufs=1, space="PSUM") as fc_pool,
        ):
            w1win = const.tile([84, 112], f16, name="w1win")
            w2r = [const.tile([112, 112], f16, tag=f"w2r{i}", name=f"w2r{i}")
                   for i in range(3)]
            # fc stationaries padded to 128 output columns: every matmul in
            # the kernel then runs with the same (128,128) PE tiling, so the
            # array never pays a tile-config switch (~100ns on each side of
            # every fc matmul otherwise)
            wfc = const.tile([112, 7, 128], f16, tag="wfc", name="wfc")
            nc.vector.memset(wfc[:], 0.0)
            b1 = const.tile([112, 1], f32, tag="b1", name="b1")
            b2 = const.tile([112, 1], f32, tag="b2", name="b2")
            bfc = const.tile([128, 1], f32, tag="bfc", name="bfc")
            # weight loads issue from the (otherwise idle) GPSIMD queue so
            # neither the SP queue (input tiles) nor the ACT queue (first
            # evictions) is blocked at startup
            for sb, dr in [(w1win, w1_d), (b1, b1_d), (w2r[0], w2r0_d),
                           (w2r[1], w2r1_d), (w2r[2], w2r2_d),
                           (b2, b2_d), (bfc, bfc_d)]:
                nc.gpsimd.dma_start(out=sb[:], in_=dr[:])
            nc.gpsimd.dma_start(out=wfc[:, :, 0:1], in_=wfc_d.rearrange(
                "p (r o) -> p r o", o=1))

            # alternate PSUM->SBUF bias+relu evictions across ACT and DVE
            evict_i = [0]

            def evict_relu(dst, src, bias):
                evict_i[0] += 1
                if evict_i[0] % 2:
                    nc.scalar.activation(dst, src, AF.Relu, bias=bias[:, 0:1])
                else:
                    nc.vector.tensor_scalar(dst, src, bias[:, 0:1], 0.0,
                                            OP.add, OP.max)

            # deferred work queue: fc matmuls / epilogues emitted at chosen
            # slots of LATER PE work so they never stall on a fresh eviction
            pending = []

            def drain(n):
                for _ in range(min(n, len(pending))):
                    pending.pop(0)()

            fcps = None
            for t in range(nt_tiles):
                # input windows split into two DMAs (halves of the oi axis)
                # so two DMA rings stream the tile's 1.2MB concurrently; the
                # first tile splits finer so conv1 row 0 starts ~2us earlier
                xt = xt_pool.tile([84, 14, TN], f16, tag="xt", name="xt")
                for h in range(2):
                    nc.sync.dma_start(
                        out=xt[:, 7 * h:7 * h + 7, :],
                        in_=xw_d[t, h].rearrange("(p o) n -> p o n", p=84))

                # ---- conv1: one shared stationary, 14 matmuls ----
                z1 = z1_pool.tile([112, 14, TN], f16, tag="z1", name="z1")
                for oi in range(14):
                    p1 = c1_pool.tile([112, TN], f32, tag="p1", name="p1")
                    nc.tensor.matmul(p1[:], w1win[:], xt[:, oi, :],
                                     start=True, stop=True)
                    evict_relu(z1[:, oi, :], p1[:], b1)
                    if oi in (2, 4, 6, 8):
                        drain(1)

                # ---- conv2 + fc ----
                z2 = z2_pool.tile([112, 7, TN], f16, tag="z2", name="z2")
                fcps = fc_pool.tile([128, TN], f32, tag="fc", name="fc")

                def fc_mm(r, fcps=fcps, z2=z2):
                    nc.tensor.matmul(fcps[:], wfc[:, r, :], z2[:, r, :],
                                     start=(r == 0), stop=(r == 6))

                for r in range(7):
                    dis = [di for di in range(3) if 0 <= 2 * r - 1 + di < 14]
                    p2 = c2_pool.tile([112, TN], f32, tag="p2", name="p2")
                    for k, di in enumerate(dis):
                        nc.tensor.matmul(p2[:], w2r[di][:],
                                         z1[:, 2 * r - 1 + di, :],
                                         start=(k == 0),
                                         stop=(k == len(dis) - 1))
                    evict_relu(z2[:, r, :], p2[:], b2)
                    pending.append(lambda r=r, f=fc_mm: f(r))
                    if r >= (1 if t == nt_tiles - 1 else 2):
                        drain(1)

                # ---- per-tile epilogue (deferred into next tile's slots) ----
                def epilogue(t=t, fcps=fcps):
                    ysb = y_pool.tile([1, TN], f32, tag="ysb", name="ysb")
                    # sigma(-t) = sigmoid(psum + (-bfc)); bias fp32
                    nc.scalar.activation(ysb[:], fcps[0:1, :], AF.Sigmoid,
                                         bias=bfc[0:1, 0:1])
                    # p = 1 - k*sigma (on GPSIMD: SBUF-only op, engine idle)
                    nc.gpsimd.tensor_scalar(ysb[:], ysb[:],
                                            -KDEC, 1.0, OP.mult, OP.add)
                    nc.sync.dma_start(out=y_d[bass.ds(t * TN, TN)],
                                      in_=ysb[0:1, :])
                pending.append(epilogue)
            drain(len(pending))

    nc.finalize()
    return nc


_NC_CACHE = {}


def _get_nc(nt_tiles):
    if nt_tiles not in _NC_CACHE:
        _NC_CACHE[nt_tiles] = _build_nc(nt_tiles)
    return _NC_CACHE[nt_tiles]


def _stage_x(x):
    """Host-side window staging: xw[core][t, h, p*7+(oi-7h), n] =
    x[core*8192 + t*512 + n, 56*oi - 28 + p], zeros when out of range."""
    x = np.asarray(x, np.float32).reshape(B, 784).astype(np.float16)
    # rows ordered (h, p, oi_local): oi = 7*h + oi_local
    h_idx = np.arange(1176) // 588
    p_idx = (np.arange(1176) % 588) // 7
    oi_idx = 7 * h_idx + (np.arange(1176) % 7)
    px = 56 * oi_idx - 28 + p_idx               # may be negative (oi=0, p<28)
    valid = px >= 0
    xg = np.zeros((B, 1176), np.float16)
    xg[:, valid] = x[:, px[valid]]
    # [B, 1176] -> [NCORES, NT, 1176, TN] -> [NCORES, NT, 2, 588, TN]
    xg = xg.reshape(NCORES, NT, TN, 1176).transpose(0, 1, 3, 2)
    return np.ascontiguousarray(xg).reshape(NCORES, NT, 2, 588, TN)


def _install_trace_hook():
    """Register the axon NTFF profiling hook (test-time only)."""
    import contextlib
    import ctypes
    import sys
    import types

    if "antenv.axon_hooks" in sys.modules:
        return
    try:
        lib = ctypes.CDLL("/opt/axon/libaxon_pjrt.so")
        if not hasattr(lib, "axon_start_nrt_profile"):
            return
        lib.axon_start_nrt_profile.argtypes = [
            ctypes.POINTER(ctypes.c_int64), ctypes.c_size_t]
        lib.axon_start_nrt_profile.restype = ctypes.c_int64
        lib.axon_stop_nrt_profile.argtypes = [ctypes.c_char_p]
        lib.axon_stop_nrt_profile.restype = ctypes.c_int64

        @contextlib.contextmanager
        def _hook(output_dir, device_ids):
            import jax
            jax.devices()
            if device_ids:
                ids = (ctypes.c_int64 * len(device_ids))(*device_ids)
                rc = lib.axon_start_nrt_profile(ids, len(device_ids))
            else:
                rc = lib.axon_start_nrt_profile(None, 0)
            if rc != 0:
                raise RuntimeError(f"axon_start_nrt_profile rc={rc}")
            try:
                yield
            finally:
                rc = lib.axon_stop_nrt_profile(output_dir.encode())
                if rc not in (0, 3):
                    raise RuntimeError(f"axon_stop_nrt_profile rc={rc}")

        mod = types.ModuleType("antenv.axon_hooks")
        mod.get_axon_ntff_profile_hook = lambda: _hook
        mod.set_axon_ntff_profile_hook = lambda h: None
        sys.modules["antenv.axon_hooks"] = mod
        import concourse.bass_utils as bu
        bu.upload_artifacts = lambda tmpdir: tmpdir
    except Exception:
        pass


def kernel(x, W1, b1, W2, b2, Wfc, bfc):
    global LAST_EXEC_NS, LAST_PROFILE_JSON
    from concourse.bass_utils import run_bass_kernel_spmd

    xw = _stage_x(x)
    W1win, W2r, wfc, b1col, b2col = _build_weight_mats(W1, b1, W2, b2, Wfc)
    bfcneg = np.full((128, 1), -np.float32(np.asarray(bfc).reshape(())),
                     np.float32)

    nc = _get_nc(NT)
    shared = {
        "w1win": W1win.astype(np.float16),
        "w2r0": np.ascontiguousarray(W2r[0]).astype(np.float16),
        "w2r1": np.ascontiguousarray(W2r[1]).astype(np.float16),
        "w2r2": np.ascontiguousarray(W2r[2]).astype(np.float16),
        "wfcneg": wfc.astype(np.float16),
        "b1col": b1col, "b2col": b2col, "bfcneg": bfcneg,
    }
    in_maps = [{"xw": xw[i], **shared} for i in range(NCORES)]
    core_ids = list(range(NCORES))
    res = run_bass_kernel_spmd(nc, in_maps, core_ids)
    y = np.concatenate([res.results[i]["y"] for i in range(NCORES)])

    if TRACE:
        _install_trace_hook()
        try:
            tres = run_bass_kernel_spmd(nc, in_maps, core_ids, trace=True)
            LAST_EXEC_NS = tres.exec_time_ns
            LAST_PROFILE_JSON = tres.profile_json
        except Exception as e:  # profiling must never break the result path
            print("trace failed:", e)

    return y.astype(np.float32)


# revision 17
# speedup vs baseline: 1.0158x; 1.0158x over previous
"""Trainium2 Bass kernel for the DiffsolClassifier model (v2).

Network (per image, NCHW fp32):
    z1 = relu(conv2d(x, W1, b1, k=3, s=2, p=1))   # [8,14,14]
    z2 = relu(conv2d(z1, W2, b2, k=3, s=2, p=1))  # [16,7,7]
    t  = flatten(z2) @ Wfc.T + bfc                # [1]
    p  = clip(1 - exp(-(softplus(t) + 1e-3)), 1e-6, 1-1e-6)
       = 1 - k*sigmoid(-t),  k = exp(-1e-3)       (clip is a no-op)

Sharding: pure data parallel, batch 65536 split 8192/core across 8 cores.

Per-core mapping (16 outer tiles x 512 images), fp16 data / fp32 PSUM:
  - Host stages x as overlapping 84-pixel conv1 windows, pixel-major:
    xw[t, h, ...] rows (p, oi) with pixel 56*oi-28+p (zeros out of
    range). Two plain contiguous DMAs per tile -> SBUF [84,14,512] fp16.
  - conv1: 14 matmuls per tile with ONE shared stationary W1win [84,112]
    (W1win[28*di + 2*oj-1+dj, co*14+oj]); bias+relu evictions to z1
    [112, 14, 512] fp16 alternate across ACT/DVE.
  - conv2: banded tap mats [112,112] x 3; rows 0..6 accumulate 2-3 taps
    in PSUM; bias+relu eviction to z2 [112, 7, 512] fp16.
  - fc: per-row stationary padded to [112,128] (negated Wfc in col 0) so
    every matmul keeps the same (128,128) PE tiling - no PE tile-config
    switches. fc matmuls and the per-tile sigmoid+affine+store epilogue
    are deferred through a pending queue into later matmul slots so the
    PE never waits on an in-flight eviction.
  - epilogue: ACT sigmoid [1,512] (sigma(-t) trick), GPSIMD affine
    p = 1 - k*sigma, SP store. Weight loads ride the GPSIMD queue.
"""

import numpy as np

B = 65536
NCORES = 8
BS = B // NCORES  # 8192 images per core
TN = 512          # images per outer tile
NT = BS // TN     # 16 outer tiles

KDEC = float(np.exp(np.float32(-0.001)))

# set by test.py for profiling; harness leaves these alone
TRACE = False
LAST_EXEC_NS = None
LAST_PROFILE_JSON = None


def _build_weight_mats(W1, b1, W2, b2, Wfc):
    """Host-side restructuring of the tiny conv/fc weights."""
    W1 = np.asarray(W1, np.float32).reshape(8, 1, 3, 3)
    W2 = np.asarray(W2, np.float32).reshape(16, 8, 3, 3)
    Wfc = np.asarray(Wfc, np.float32).reshape(1, 784)

    # W1win[w, (co,oj)] over an 84-pixel window, w = 28*di + (2*oj-1+dj)
    W1win = np.zeros((84, 112), np.float32)
    for co in range(8):
        for oj in range(14):
            m = co * 14 + oj
            for di in range(3):
                for dj in range(3):
                    j = 2 * oj - 1 + dj
                    if 0 <= j < 28:
                        W1win[28 * di + j, m] = W1[co, 0, di, dj]

    # conv2 tap matrices: W2r[di][(ci,j), (co2,oj2)]
    W2r = np.zeros((3, 112, 112), np.float32)
    for di in range(3):
        for co in range(16):
            for oj in range(7):
                m = co * 7 + oj
                for ci in range(8):
                    for dj in range(3):
                        j = 2 * oj - 1 + dj
                        if 0 <= j < 14:
                            W2r[di, ci * 14 + j, m] = W2[co, ci, di, dj]

    # fc columns per z2 row, negated (p = 1 - k*sigmoid(-t) trick)
    wfc = np.zeros((112, 7), np.float32)
    for co in range(16):
        for i2 in range(7):
            for oj in range(7):
                wfc[co * 7 + oj, i2] = -Wfc[0, co * 49 + i2 * 7 + oj]

    b1col = np.repeat(np.asarray(b1, np.float32), 14).reshape(112, 1)
    b2col = np.repeat(np.asarray(b2, np.float32), 7).reshape(112, 1)
    return W1win, W2r, wfc, b1col, b2col


def _build_nc(nt_tiles):
    import concourse.bacc as bacc
    import concourse.bass as bass
    import concourse.mybir as mybir
    import concourse.tile as tile

    f32 = mybir.dt.float32
    f16 = mybir.dt.float16
    AF = mybir.ActivationFunctionType
    OP = mybir.AluOpType
    bs = nt_tiles * TN
    ngrp = (nt_tiles + 3) // 4

    nc = bacc.Bacc(None)
    xw_d = nc.declare_dram_parameter("xw", [nt_tiles, 2, 588, TN], f16,
                                     isOutput=False)
    w1_d = nc.declare_dram_parameter("w1win", [84, 112], f16, isOutput=False)
    w2r0_d = nc.declare_dram_parameter("w2r0", [112, 112], f16, isOutput=False)
    w2r1_d = nc.declare_dram_parameter("w2r1", [112, 112], f16, isOutput=False)
    w2r2_d = nc.declare_dram_parameter("w2r2", [112, 112], f16, isOutput=False)
    wfc_d = nc.declare_dram_parameter("wfcneg", [112, 7], f16, isOutput=False)
    b1_d = nc.declare_dram_parameter("b1col", [112, 1], f32, isOutput=False)
    b2_d = nc.declare_dram_parameter("b2col", [112, 1], f32, isOutput=False)
    bfc_d = nc.declare_dram_parameter("bfcneg", [128, 1], f32, isOutput=False)
    y_d = nc.declare_dram_parameter("y", [bs], f32, isOutput=True)

    with tile.TileContext(nc) as tc:
        with (
            tc.tile_pool(name="const", bufs=1) as const,
            tc.tile_pool(name="xt_pool", bufs=4) as xt_pool,
            tc.tile_pool(name="z1_pool", bufs=3) as z1_pool,
            tc.tile_pool(name="z2_pool", bufs=2) as z2_pool,
            tc.tile_pool(name="y_pool", bufs=2) as y_pool,
            tc.tile_pool(name="c1_psum", bufs=4, space="PSUM") as c1_pool,
            tc.tile_pool(name="c2_psum", bufs=3, space="PSUM") as c2_pool,
            tc.tile_pool(name="fc_psum", bufs=1, space="PSUM") as fc_pool,
        ):
            w1win = const.tile([84, 112], f16, name="w1win")
            w2r = [const.tile([112, 112], f16, tag=f"w2r{i}", name=f"w2r{i}")
                   for i in range(3)]
            # fc stationaries padded to 128 output columns: every matmul in
            # the kernel then runs with the same (128,128) PE tiling, so the
            # array never pays a tile-config switch (~100ns on each side of
            # every fc matmul otherwise)
            wfc = const.tile([112, 7, 128], f16, tag="wfc", name="wfc")
            nc.vector.memset(wfc[:], 0.0)
            b1 = const.tile([112, 1], f32, tag="b1", name="b1")
            b2 = const.tile([112, 1], f32, tag="b2", name="b2")
            bfc = const.tile([128, 1], f32, tag="bfc", name="bfc")
            # weight loads issue from the (otherwise idle) GPSIMD queue so
            # neither the SP queue (input tiles) nor the ACT queue (first
            # evictions) is blocked at startup
            for sb, dr in [(w1win, w1_d), (b1, b1_d), (w2r[0], w2r0_d),
                           (w2r[1], w2r1_d), (w2r[2], w2r2_d),
                           (b2, b2_d), (bfc, bfc_d)]:
                nc.gpsimd.dma_start(out=sb[:], in_=dr[:])
            nc.gpsimd.dma_start(out=wfc[:, :, 0:1], in_=wfc_d.rearrange(
                "p (r o) -> p r o", o=1))

            # alternate PSUM->SBUF bias+relu evictions across ACT and DVE
            evict_i = [0]

            def evict_relu(dst, src, bias):
                evict_i[0] += 1
                if evict_i[0] % 2:
                    nc.scalar.activation(dst, src, AF.Relu, bias=bias[:, 0:1])
                else:
                    nc.vector.tensor_scalar(dst, src, bias[:, 0:1], 0.0,
                                            OP.add, OP.max)

            # deferred work queue: fc matmuls / epilogues emitted at chosen
            # slots of LATER PE work so they never stall on a fresh eviction
            pending = []

            def drain(n):
                for _ in range(min(n, len(pending))):
                    pending.pop(0)()

            fcps = None
            for t in range(nt_tiles):
                # input windows split into two DMAs (halves of the oi axis)
                # so two DMA rings stream the tile's 1.2MB concurrently; the
                # first tile splits finer so conv1 row 0 starts ~2us earlier
                xt = xt_pool.tile([84, 14, TN], f16, tag="xt", name="xt")
                for h in range(2):
                    nc.sync.dma_start(
                        out=xt[:, 7 * h:7 * h + 7, :],
                        in_=xw_d[t, h].rearrange("(p o) n -> p o n", p=84))

                # ---- conv1: one shared stationary, 14 matmuls ----
                z1 = z1_pool.tile([112, 14, TN], f16, tag="z1", name="z1")
                for oi in range(14):
                    p1 = c1_pool.tile([112, TN], f32, tag="p1", name="p1")
                    nc.tensor.matmul(p1[:], w1win[:], xt[:, oi, :],
                                     start=True, stop=True)
                    evict_relu(z1[:, oi, :], p1[:], b1)
                    if oi in (2, 4, 6, 8):
                        drain(1)

                # ---- conv2 + fc ----
                z2 = z2_pool.tile([112, 7, TN], f16, tag="z2", name="z2")
                fcps = fc_pool.tile([128, TN], f32, tag="fc", name="fc")

                def fc_mm(r, fcps=fcps, z2=z2):
                    nc.tensor.matmul(fcps[:], wfc[:, r, :], z2[:, r, :],
                                     start=(r == 0), stop=(r == 6))

                for r in range(7):
                    dis = [di for di in range(3) if 0 <= 2 * r - 1 + di < 14]
                    p2 = c2_pool.tile([112, TN], f32, tag="p2", name="p2")
                    for k, di in enumerate(dis):
                        nc.tensor.matmul(p2[:], w2r[di][:],
                                         z1[:, 2 * r - 1 + di, :],
                                         start=(k == 0),
                                         stop=(k == len(dis) - 1))
                    evict_relu(z2[:, r, :], p2[:], b2)
                    pending.append(lambda r=r, f=fc_mm: f(r))
                    drain(1) if r >= 2 else None

                # ---- per-tile epilogue (deferred into next tile's slots) ----
                def epilogue(t=t, fcps=fcps):
                    ysb = y_pool.tile([1, TN], f32, tag="ysb", name="ysb")
                    # sigma(-t) = sigmoid(psum + (-bfc)); bias fp32
                    nc.scalar.activation(ysb[:], fcps[0:1, :], AF.Sigmoid,
                                         bias=bfc[0:1, 0:1])
                    # p = 1 - k*sigma (on GPSIMD: SBUF-only op, engine idle)
                    nc.gpsimd.tensor_scalar(ysb[:], ysb[:],
                                            -KDEC, 1.0, OP.mult, OP.add)
                    nc.sync.dma_start(out=y_d[bass.ds(t * TN, TN)],
                                      in_=ysb[0:1, :])
                pending.append(epilogue)
            drain(len(pending))

    nc.finalize()
    return nc


_NC_CACHE = {}


def _get_nc(nt_tiles):
    if nt_tiles not in _NC_CACHE:
        _NC_CACHE[nt_tiles] = _build_nc(nt_tiles)
    return _NC_CACHE[nt_tiles]


def _stage_x(x):
    """Host-side window staging: xw[core][t, h, p*7+(oi-7h), n] =
    x[core*8192 + t*512 + n, 56*oi - 28 + p], zeros when out of range."""
    x = np.asarray(x, np.float32).reshape(B, 784).astype(np.float16)
    # rows ordered (h, p, oi_local): oi = 7*h + oi_local
    h_idx = np.arange(1176) // 588
    p_idx = (np.arange(1176) % 588) // 7
    oi_idx = 7 * h_idx + (np.arange(1176) % 7)
    px = 56 * oi_idx - 28 + p_idx               # may be negative (oi=0, p<28)
    valid = px >= 0
    xg = np.zeros((B, 1176), np.float16)
    xg[:, valid] = x[:, px[valid]]
    # [B, 1176] -> [NCORES, NT, 1176, TN] -> [NCORES, NT, 2, 588, TN]
    xg = xg.reshape(NCORES, NT, TN, 1176).transpose(0, 1, 3, 2)
    return np.ascontiguousarray(xg).reshape(NCORES, NT, 2, 588, TN)


def _install_trace_hook():
    """Register the axon NTFF profiling hook (test-time only)."""
    import contextlib
    import ctypes
    import sys
    import types

    if "antenv.axon_hooks" in sys.modules:
        return
    try:
        lib = ctypes.CDLL("/opt/axon/libaxon_pjrt.so")
        if not hasattr(lib, "axon_start_nrt_profile"):
            return
        lib.axon_start_nrt_profile.argtypes = [
            ctypes.POINTER(ctypes.c_int64), ctypes.c_size_t]
        lib.axon_start_nrt_profile.restype = ctypes.c_int64
        lib.axon_stop_nrt_profile.argtypes = [ctypes.c_char_p]
        lib.axon_stop_nrt_profile.restype = ctypes.c_int64

        @contextlib.contextmanager
        def _hook(output_dir, device_ids):
            import jax
            jax.devices()
            if device_ids:
                ids = (ctypes.c_int64 * len(device_ids))(*device_ids)
                rc = lib.axon_start_nrt_profile(ids, len(device_ids))
            else:
                rc = lib.axon_start_nrt_profile(None, 0)
            if rc != 0:
                raise RuntimeError(f"axon_start_nrt_profile rc={rc}")
            try:
                yield
            finally:
                rc = lib.axon_stop_nrt_profile(output_dir.encode())
                if rc not in (0, 3):
                    raise RuntimeError(f"axon_stop_nrt_profile rc={rc}")

        mod = types.ModuleType("antenv.axon_hooks")
        mod.get_axon_ntff_profile_hook = lambda: _hook
        mod.set_axon_ntff_profile_hook = lambda h: None
        sys.modules["antenv.axon_hooks"] = mod
        import concourse.bass_utils as bu
        bu.upload_artifacts = lambda tmpdir: tmpdir
    except Exception:
        pass


def kernel(x, W1, b1, W2, b2, Wfc, bfc):
    global LAST_EXEC_NS, LAST_PROFILE_JSON
    from concourse.bass_utils import run_bass_kernel_spmd

    xw = _stage_x(x)
    W1win, W2r, wfc, b1col, b2col = _build_weight_mats(W1, b1, W2, b2, Wfc)
    bfcneg = np.full((128, 1), -np.float32(np.asarray(bfc).reshape(())),
                     np.float32)

    nc = _get_nc(NT)
    shared = {
        "w1win": W1win.astype(np.float16),
        "w2r0": np.ascontiguousarray(W2r[0]).astype(np.float16),
        "w2r1": np.ascontiguousarray(W2r[1]).astype(np.float16),
        "w2r2": np.ascontiguousarray(W2r[2]).astype(np.float16),
        "wfcneg": wfc.astype(np.float16),
        "b1col": b1col, "b2col": b2col, "bfcneg": bfcneg,
    }
    in_maps = [{"xw": xw[i], **shared} for i in range(NCORES)]
    core_ids = list(range(NCORES))
    res = run_bass_kernel_spmd(nc, in_maps, core_ids)
    y = np.concatenate([res.results[i]["y"] for i in range(NCORES)])

    if TRACE:
        _install_trace_hook()
        try:
            tres = run_bass_kernel_spmd(nc, in_maps, core_ids, trace=True)
            LAST_EXEC_NS = tres.exec_time_ns
            LAST_PROFILE_JSON = tres.profile_json
        except Exception as e:  # profiling must never break the result path
            print("trace failed:", e)

    return y.astype(np.float32)
